# revision 11
# baseline (speedup 1.0000x reference)
"""Multi-head attention forward (B=8, N=1024, C=768, H=12) on 8 TRN2 NeuronCores.

Sharding: data-parallel over batch — core b computes batch b end-to-end
(weights replicated, no collectives). All matmuls bf16 with fp32 PSUM.

v6 design:
  - x transposed on the host; x^T tiles DMA straight into SBUF (no PE
    transposes or PSUM evacuation for x^T).
  - S^T = k^T q: the two heads of a pair run as tile_position (0,0)/(64,0)
    matmuls into the TWO BANKS OF ONE PSUM TILE, so a single exp frees
    both banks together and the pair streams concurrently (2x) — with
    separate tiles the two sequential exps serialize the pair.
  - E tiles hold [E_headA(jb) | E_headB(jb)]; PV uses the baseline
    race-free per-head scheme: stationary [v_h | ones] (65 cols), psum
    [65,512] accumulating over key blocks; row 64 is the softmax
    denominator.
  - PV runs one jp behind S/exp so the in-order PE queue doesn't
    head-of-line block on ACT; spare PE slots host woven qk matmuls for
    later pairs, v matmuls, and the ih=0 half of the projection.
  - HAM warmup dummies bridge engine-boot to first data; DMA ordered
    (w_qk[cc], xT[cc]) x6 -> w_v -> w_proj with pair-0 qkv matmuls
    accumulating cc-progressively as chunks land.
"""
import numpy as np
from contextlib import ExitStack

import concourse.bacc as bacc
import concourse.tile as tile
from concourse import mybir, bass_utils

F32 = mybir.dt.float32
BF16 = mybir.dt.bfloat16
EXP = mybir.ActivationFunctionType.Exp

B = 8
N = 1024       # sequence length
C = 768        # channels
H = 12         # heads
HD = 64        # head dim
NB = N // 128  # 8 seq blocks
CB = C // 128  # 6 channel chunks
HP = H // 2    # 6 head pairs
VW = HD + 1    # 65: v columns per head incl. ones column
SCALE = float(HD) ** -0.5
NDUM = 6       # HAM warmup dummy matmuls

_NC = None


def _build():
    nc = bacc.Bacc("TRN2", target_bir_lowering=False, debug=False, num_devices=B)
    xT = nc.dram_tensor("x", [C, N], BF16, kind="ExternalInput")  # host-transposed
    w_qkv = nc.dram_tensor("w_qkv", [C, 3 * C], BF16, kind="ExternalInput")
    w_proj = nc.dram_tensor("w_proj", [C, C], BF16, kind="ExternalInput")
    b_proj = nc.dram_tensor("b_proj", [1, C], F32, kind="ExternalInput")
    y = nc.dram_tensor("y", [N, C], F32, kind="ExternalOutput")

    with tile.TileContext(nc) as tc, ExitStack() as ctx:
        const = ctx.enter_context(tc.tile_pool(name="const", bufs=1))
        p_xT = ctx.enter_context(tc.tile_pool(name="p_xT", bufs=1))
        p_wq = ctx.enter_context(tc.tile_pool(name="p_wq", bufs=1))
        p_qk = ctx.enter_context(tc.tile_pool(name="p_qk", bufs=1))
        p_v = ctx.enter_context(tc.tile_pool(name="p_v", bufs=1))
        p_out = ctx.enter_context(tc.tile_pool(name="p_out", bufs=1))
        p_wp = ctx.enter_context(tc.tile_pool(name="p_wp", bufs=1))

        dummyS = const.tile([128, 128], BF16, tag="dummyS")
        nc.vector.memset(dummyS[:], 0.0)
        dummyM = const.tile([128, 512], BF16, tag="dummyM")
        nc.vector.memset(dummyM[:], 0.0)
        act_warm = const.tile([128, 8], F32, tag="act_warm")
        nc.scalar.activation(act_warm[:], dummyS[:, 0:8], EXP, scale=1.0)
        ones12 = const.tile([128, H], F32, tag="ones12")
        nc.vector.memset(ones12[:], 1.0)
        bias_row = const.tile([1, C], F32, tag="bias_row")
        bias_bc = const.tile([128, C], F32, tag="bias_bc")

        xt = [p_xT.tile([128, N], BF16, tag=f"xt{c}", name=f"xt{c}") for c in range(CB)]
        wq = [p_wq.tile([128, 3 * C], BF16, tag=f"wq{c}", name=f"wq{c}") for c in range(CB)]
        qT = [p_qk.tile([128, N], BF16, tag=f"qT{t}", name=f"qT{t}") for t in range(HP)]
        kT = [p_qk.tile([128, N], BF16, tag=f"kT{t}", name=f"kT{t}") for t in range(HP)]
        vn = [p_v.tile([128, H * VW], BF16, tag=f"v{ib}", name=f"v{ib}") for ib in range(NB)]
        outT = [p_out.tile([128, N], BF16, tag=f"outT{t}", name=f"outT{t}") for t in range(HP)]
        wp = [p_wp.tile([128, C], BF16, tag=f"wp{c}", name=f"wp{c}") for c in range(CB)]

        # ---- input DMA, ordered by need
        nc.sync.dma_start(bias_row[:], b_proj.ap())
        for cc in range(CB):
            nc.sync.dma_start(wq[cc][:, 0:2 * C],
                              w_qkv.ap()[cc * 128:(cc + 1) * 128, 0:2 * C])
            nc.sync.dma_start(xt[cc][:], xT.ap()[cc * 128:(cc + 1) * 128, :])
        for cc in range(CB):
            nc.sync.dma_start(wq[cc][:, 2 * C:3 * C],
                              w_qkv.ap()[cc * 128:(cc + 1) * 128, 2 * C:3 * C])
        for cc in range(CB):
            nc.sync.dma_start(wp[cc][:], w_proj.ap()[cc * 128:(cc + 1) * 128, :])
        nc.gpsimd.partition_broadcast(bias_bc[:], bias_row[:])

        with (
            tc.tile_pool(name="p_E", bufs=8) as p_E,
            tc.tile_pool(name="p_nrm", bufs=2) as p_nrm,
            tc.tile_pool(name="p_y", bufs=2) as p_y,
            tc.tile_pool(name="ps_s", bufs=2, space="PSUM") as ps_s,
            tc.tile_pool(name="ps_pv", bufs=2, space="PSUM") as ps_pv,
            tc.tile_pool(name="ps_w", bufs=2, space="PSUM") as ps_w,
        ):
            # ---- HAM warmup: junk matmuls bridge engine boot -> first data
            for i in range(NDUM):
                pd = ps_w.tile([128, 512], F32, tag="w", name=f"dum{i}")
                nc.tensor.matmul(pd[:], dummyS[:], dummyM[:], start=True, stop=True)

            # ---- qkv helpers
            def emit_qk_quarter(t, f_idx, nh):
                s_ = ps_s.tile([128, 512], F32, tag="s", name=f"qk{t}_{f_idx}{nh}")
                for cc in range(CB):
                    nc.tensor.matmul(
                        s_[:],
                        wq[cc][:, f_idx * C + t * 128: f_idx * C + (t + 1) * 128],
                        xt[cc][:, nh * 512:(nh + 1) * 512],
                        start=(cc == 0), stop=(cc == CB - 1))
                dst = (qT, kT)[f_idx][t]
                nc.vector.tensor_copy(dst[:, nh * 512:(nh + 1) * 512], s_[:])

            def emit_qk0_ccprog():
                s0 = ps_s.tile([128, 1024], F32, tag="s", name="qk0_q")
                s1 = ps_s.tile([128, 1024], F32, tag="s", name="qk0_k")
                for cc in range(CB):
                    for f_idx, s_ in ((0, s0), (1, s1)):
                        for nh in (0, 1):
                            nc.tensor.matmul(
                                s_[:, nh * 512:(nh + 1) * 512],
                                wq[cc][:, f_idx * C: f_idx * C + 128],
                                xt[cc][:, nh * 512:(nh + 1) * 512],
                                start=(cc == 0), stop=(cc == CB - 1),
                                skip_group_check=True)
                nc.vector.tensor_copy(qT[0][:], s0[:])
                nc.vector.tensor_copy(kT[0][:], s1[:])

            def emit_v_group(ib, half):
                pv = ps_w.tile([128, 384], F32, tag="w", name=f"pv{ib}_{half}")
                for cc in range(CB):
                    nc.tensor.matmul(
                        pv[:],
                        xt[cc][:, ib * 128:(ib + 1) * 128],
                        wq[cc][:, 2 * C + half * 384: 2 * C + (half + 1) * 384],
                        start=(cc == 0), stop=(cc == CB - 1))
                nc.vector.tensor_copy(
                    vn[ib][:, half * 6 * VW:(half + 1) * 6 * VW]
                    .rearrange("p (h d) -> p h d", d=VW)[:, :, 0:HD],
                    pv[:].rearrange("p (h d) -> p h d", d=HD))
                if half == 1:
                    nc.vector.tensor_copy(
                        vn[ib][:].rearrange("p (h d) -> p h d", d=VW)[:, :, HD:VW],
                        ones12[:])

            ys = [None] * NB

            def emit_proj_half(nb, cp):
                py = ps_s.tile([128, 384], F32, tag="s", name=f"py{nb}_{cp}")
                for t2 in range(CB):
                    nc.tensor.matmul(
                        py[:], outT[t2][:, nb * 128:(nb + 1) * 128],
                        wp[t2][:, cp * 384:(cp + 1) * 384],
                        start=(t2 == 0), stop=(t2 == CB - 1))
                if ys[nb] is None:
                    ys[nb] = p_y.tile([128, C], F32, tag="ys", name=f"ys{nb}")
                nc.vector.tensor_add(
                    ys[nb][:, cp * 384:(cp + 1) * 384], py[:],
                    bias_bc[:, cp * 384:(cp + 1) * 384])

            def emit_y_dma(nb):
                nc.sync.dma_start(y.ap()[nb * 128:(nb + 1) * 128, :], ys[nb][:])

            # ---- prologue: qk for pairs 0,1 and v for blocks 0,1
            emit_qk0_ccprog()
            for f_idx in (0, 1):
                for nh in (0, 1):
                    emit_qk_quarter(1, f_idx, nh)
            for ib in (0, 1):
                for half in (0, 1):
                    emit_v_group(ib, half)

            # ---- weave plan
            weave = {}

            def add_weave(t, ih, jp, thunk):
                weave.setdefault((t, ih, jp), []).append(thunk)

            def pop_weave(t, ih, jp, n):
                lst = weave.get((t, ih, jp), [])
                for _ in range(n):
                    if not lst:
                        return
                    lst.pop(0)()

            # v blocks 2..7 early in (t=0, ih=0)
            for k in range(6):
                ib = k + 2
                for half in (0, 1):
                    add_weave(0, 0, min(3, k // 2), (lambda ib=ib, half=half:
                                                     emit_v_group(ib, half)))
            # qk pairs 2..5, four quarter-chunks each
            qslots = {
                2: [(0, 0, 3), (0, 1, 0), (0, 1, 1), (0, 1, 2)],
                3: [(0, 1, 3), (1, 0, 0), (1, 0, 1), (1, 0, 2)],
                4: [(1, 0, 3), (1, 1, 0), (1, 1, 1), (1, 1, 2)],
                5: [(1, 1, 3), (2, 0, 0), (2, 0, 1), (2, 0, 2)],
            }
            for t, slots in qslots.items():
                qi = 0
                for f_idx in (0, 1):
                    for nh in (0, 1):
                        ts_, ihs, jps = slots[qi]
                        add_weave(ts_, ihs, jps,
                                  (lambda t=t, f=f_idx, n=nh:
                                   emit_qk_quarter(t, f, n)))
                        qi += 1
            # proj for the ih=0 query half woven into (t=5, ih=1)
            for nb in range(4):
                for cp in (0, 1):
                    add_weave(5, 1, nb, (lambda nb=nb, cp=cp:
                                         emit_proj_half(nb, cp)))
                add_weave(5, 1, nb, (lambda nb=nb: emit_y_dma(nb)))

            # ---- attention: per head pair, PV lagged one jp
            for t in range(HP):
                hA, hB = 2 * t, 2 * t + 1
                for ih in (0, 1):
                    ppA = ps_pv.tile([VW, 512], F32, tag="pv", name=f"ppA{t}{ih}")
                    ppB = ps_pv.tile([VW, 512], F32, tag="pv", name=f"ppB{t}{ih}")

                    def emit_pv(jp, es):
                        # es[jbi] = [E_A(jb) | E_B(jb)] for jb = 2jp+jbi
                        for jbi, jb in enumerate((2 * jp, 2 * jp + 1)):
                            eU = es[jbi]
                            st = jp == 0 and jbi == 0
                            sp = jp == 3 and jbi == 1
                            nc.tensor.matmul(
                                ppA[:], vn[jb][:, hA * VW:(hA + 1) * VW],
                                eU[:, 0:512], start=st, stop=sp)
                            nc.tensor.matmul(
                                ppB[:], vn[jb][:, hB * VW:(hB + 1) * VW],
                                eU[:, 512:1024], start=st, stop=sp)

                    prev = None
                    for jp in range(4):
                        jbs = (2 * jp, 2 * jp + 1)
                        es = []
                        for jbi, jb in enumerate(jbs):
                            # pair writes the two banks of ONE tile
                            sU = ps_s.tile([128, 1024], F32, tag="s",
                                           name=f"sU{t}{ih}{jp}{jb}")
                            nc.tensor.matmul(
                                sU[:, 0:512],
                                kT[t][0:64, jb * 128:(jb + 1) * 128],
                                qT[t][0:64, ih * 512:(ih + 1) * 512],
                                start=True, stop=True, tile_position=(0, 0))
                            nc.tensor.matmul(
                                sU[:, 512:1024],
                                kT[t][64:128, jb * 128:(jb + 1) * 128],
                                qT[t][64:128, ih * 512:(ih + 1) * 512],
                                start=True, stop=True, tile_position=(64, 0))
                            eU = p_E.tile([128, 1024], BF16, tag="e",
                                          name=f"e{t}{ih}{jp}{jb}")
                            nc.scalar.activation(eU[:], sU[:], EXP, scale=SCALE)
                            es.append(eU)
                            if jbi == 0:
                                pop_weave(t, ih, jp, 1)
                        if prev is not None:
                            emit_pv(prev[0], prev[1])
                        prev = (jp, es)
                        pop_weave(t, ih, jp, 99)
                    emit_pv(prev[0], prev[1])

                    # ---- normalize: out^T = PV[0:64] / rowsum (row 64)
                    for pp, po in ((ppA, 0), (ppB, 64)):
                        rs = p_nrm.tile([1, 512], F32, tag="rs",
                                        name=f"rs{t}{ih}{po}")
                        nc.vector.tensor_copy(rs[:], pp[HD:VW, :])
                        bc = p_nrm.tile([64, 512], F32, tag="bc",
                                        name=f"bc{t}{ih}{po}")
                        nc.gpsimd.partition_broadcast(bc[:], rs[:])
                        rc = p_nrm.tile([64, 512], F32, tag="rc",
                                        name=f"rc{t}{ih}{po}")
                        nc.vector.reciprocal_approx_fast(rc[:], bc[:])
                        if po == 0:
                            nc.vector.tensor_mul(
                                outT[t][0:64, ih * 512:(ih + 1) * 512],
                                pp[0:HD, :], rc[:])
                        else:
                            ob = p_nrm.tile([64, 512], BF16, tag="ob",
                                            name=f"ob{t}{ih}")
                            nc.vector.tensor_mul(ob[:], pp[0:HD, :], rc[:])
                            nc.sync.dma_start(
                                outT[t][64:128, ih * 512:(ih + 1) * 512], ob[:])

            # ---- proj tail: ih=1 query half + remaining stores
            for nb in range(4, NB):
                for cp in (0, 1):
                    emit_proj_half(nb, cp)
                emit_y_dma(nb)

    nc.compile()
    return nc


def _get_nc():
    global _NC
    if _NC is None:
        _NC = _build()
    return _NC


def _run(in_maps, trace=False, tmpdir=None):
    return bass_utils.run_bass_kernel_spmd(
        _get_nc(), in_maps, core_ids=list(range(B)), trace=trace, tmpdir=tmpdir)


def _in_maps(x, w_qkv, w_proj, b_proj):
    import ml_dtypes
    bf = ml_dtypes.bfloat16
    x = np.asarray(x, dtype=np.float32).astype(bf)
    w_qkv = np.ascontiguousarray(np.asarray(w_qkv, dtype=np.float32).astype(bf))
    w_proj = np.ascontiguousarray(np.asarray(w_proj, dtype=np.float32).astype(bf))
    b_proj = np.ascontiguousarray(np.asarray(b_proj, dtype=np.float32)).reshape(1, C)
    return [
        {"x": np.ascontiguousarray(x[b].T), "w_qkv": w_qkv,
         "w_proj": w_proj, "b_proj": b_proj}
        for b in range(B)
    ]


def kernel(x, w_qkv, w_proj, b_proj):
    res = _run(_in_maps(x, w_qkv, w_proj, b_proj))
    return np.stack([res.results[b]["y"] for b in range(B)], axis=0)


# revision 15
# speedup vs baseline: 1.1278x; 1.1278x over previous
"""Multi-head attention forward (B=8, N=1024, C=768, H=12) on 8 TRN2 NeuronCores.

Sharding: data-parallel over batch — core b computes batch b end-to-end
(weights replicated, no collectives). All matmuls bf16 with fp32 PSUM.

v6 design:
  - x transposed on the host; x^T tiles DMA straight into SBUF (no PE
    transposes or PSUM evacuation for x^T).
  - S^T = k^T q: the two heads of a pair run as tile_position (0,0)/(64,0)
    matmuls into the TWO BANKS OF ONE PSUM TILE, so a single exp frees
    both banks together and the pair streams concurrently (2x) — with
    separate tiles the two sequential exps serialize the pair.
  - E tiles hold [E_headA(jb) | E_headB(jb)]; PV uses the baseline
    race-free per-head scheme: stationary [v_h | ones] (65 cols), psum
    [65,512] accumulating over key blocks; row 64 is the softmax
    denominator.
  - PV runs one jp behind S/exp so the in-order PE queue doesn't
    head-of-line block on ACT; spare PE slots host woven qk matmuls for
    later pairs, v matmuls, and the ih=0 half of the projection.
  - HAM warmup dummies bridge engine-boot to first data; DMA ordered
    (w_qk[cc], xT[cc]) x6 -> w_v -> w_proj with pair-0 qkv matmuls
    accumulating cc-progressively as chunks land.
"""
import numpy as np
from contextlib import ExitStack

import concourse.bacc as bacc
import concourse.tile as tile
from concourse import mybir, bass_utils

F32 = mybir.dt.float32
BF16 = mybir.dt.bfloat16
EXP = mybir.ActivationFunctionType.Exp

B = 8
N = 1024       # sequence length
C = 768        # channels
H = 12         # heads
HD = 64        # head dim
NB = N // 128  # 8 seq blocks
CB = C // 128  # 6 channel chunks
HP = H // 2    # 6 head pairs
VW = HD + 1    # 65: v columns per head incl. ones column
SCALE = float(HD) ** -0.5
NDUM = 14      # HAM warmup dummy matmuls

_NC = None


def _build():
    nc = bacc.Bacc("TRN2", target_bir_lowering=False, debug=False, num_devices=B)
    xT = nc.dram_tensor("x", [C, N], BF16, kind="ExternalInput")  # host-transposed
    w_qkv = nc.dram_tensor("w_qkv", [C, 3 * C], BF16, kind="ExternalInput")
    w_proj = nc.dram_tensor("w_proj", [C, C], BF16, kind="ExternalInput")
    b_proj = nc.dram_tensor("b_proj", [1, C], F32, kind="ExternalInput")
    y = nc.dram_tensor("y", [N, C], F32, kind="ExternalOutput")

    with tile.TileContext(nc) as tc, ExitStack() as ctx:
        const = ctx.enter_context(tc.tile_pool(name="const", bufs=1))
        p_xT = ctx.enter_context(tc.tile_pool(name="p_xT", bufs=1))
        p_wq = ctx.enter_context(tc.tile_pool(name="p_wq", bufs=1))
        p_qk = ctx.enter_context(tc.tile_pool(name="p_qk", bufs=1))
        p_v = ctx.enter_context(tc.tile_pool(name="p_v", bufs=1))
        p_out = ctx.enter_context(tc.tile_pool(name="p_out", bufs=1))
        p_wp = ctx.enter_context(tc.tile_pool(name="p_wp", bufs=1))

        dummyS = const.tile([128, 128], BF16, tag="dummyS")
        nc.vector.memset(dummyS[:], 0.0)
        dummyM = const.tile([128, 512], BF16, tag="dummyM")
        nc.vector.memset(dummyM[:], 0.0)
        act_warm = const.tile([128, 8], F32, tag="act_warm")
        nc.scalar.activation(act_warm[:], dummyS[:, 0:8], EXP, scale=1.0)
        ones12 = const.tile([128, H], F32, tag="ones12")
        nc.vector.memset(ones12[:], 1.0)
        bias_row = const.tile([1, C], F32, tag="bias_row")
        bias_bc = const.tile([128, C], F32, tag="bias_bc")

        xt = [p_xT.tile([128, N], BF16, tag=f"xt{c}", name=f"xt{c}") for c in range(CB)]
        wq = [p_wq.tile([128, 3 * C], BF16, tag=f"wq{c}", name=f"wq{c}") for c in range(CB)]
        qT = [p_qk.tile([128, N], BF16, tag=f"qT{t}", name=f"qT{t}") for t in range(HP)]
        kT = [p_qk.tile([128, N], BF16, tag=f"kT{t}", name=f"kT{t}") for t in range(HP)]
        vn = [p_v.tile([128, H * VW], BF16, tag=f"v{ib}", name=f"v{ib}") for ib in range(NB)]
        outT = [p_out.tile([128, N], BF16, tag=f"outT{t}", name=f"outT{t}") for t in range(HP)]
        wp = [p_wp.tile([128, C], BF16, tag=f"wp{c}", name=f"wp{c}") for c in range(CB)]

        # ---- input DMA, ordered by need
        nc.sync.dma_start(bias_row[:], b_proj.ap())
        for cc in range(CB):
            nc.sync.dma_start(wq[cc][:, 0:2 * C],
                              w_qkv.ap()[cc * 128:(cc + 1) * 128, 0:2 * C])
            nc.sync.dma_start(xt[cc][:], xT.ap()[cc * 128:(cc + 1) * 128, :])
        for cc in range(CB):
            nc.sync.dma_start(wq[cc][:, 2 * C:3 * C],
                              w_qkv.ap()[cc * 128:(cc + 1) * 128, 2 * C:3 * C])
        for cc in range(CB):
            nc.sync.dma_start(wp[cc][:], w_proj.ap()[cc * 128:(cc + 1) * 128, :])
        nc.gpsimd.partition_broadcast(bias_bc[:], bias_row[:])

        with (
            tc.tile_pool(name="p_E", bufs=8) as p_E,
            tc.tile_pool(name="p_nrm", bufs=2) as p_nrm,
            tc.tile_pool(name="p_y", bufs=2) as p_y,
            tc.tile_pool(name="ps_s", bufs=2, space="PSUM") as ps_s,
            tc.tile_pool(name="ps_pv", bufs=2, space="PSUM") as ps_pv,
            tc.tile_pool(name="ps_w", bufs=2, space="PSUM") as ps_w,
        ):
            # ---- HAM warmup: junk matmuls bridge engine boot -> first data
            for i in range(NDUM):
                pd = ps_w.tile([128, 512], F32, tag="w", name=f"dum{i}")
                nc.tensor.matmul(pd[:], dummyS[:], dummyM[:], start=True, stop=True)

            # ---- qkv helpers
            def emit_qk_quarter(t, f_idx, nh):
                s_ = ps_s.tile([128, 512], F32, tag="s", name=f"qk{t}_{f_idx}{nh}")
                for cc in range(CB):
                    nc.tensor.matmul(
                        s_[:],
                        wq[cc][:, f_idx * C + t * 128: f_idx * C + (t + 1) * 128],
                        xt[cc][:, nh * 512:(nh + 1) * 512],
                        start=(cc == 0), stop=(cc == CB - 1))
                dst = (qT, kT)[f_idx][t]
                nc.vector.tensor_copy(dst[:, nh * 512:(nh + 1) * 512], s_[:])

            def emit_qk0_ccprog():
                s0 = ps_s.tile([128, 1024], F32, tag="s", name="qk0_q")
                s1 = ps_s.tile([128, 1024], F32, tag="s", name="qk0_k")
                for cc in range(CB):
                    for f_idx, s_ in ((0, s0), (1, s1)):
                        for nh in (0, 1):
                            nc.tensor.matmul(
                                s_[:, nh * 512:(nh + 1) * 512],
                                wq[cc][:, f_idx * C: f_idx * C + 128],
                                xt[cc][:, nh * 512:(nh + 1) * 512],
                                start=(cc == 0), stop=(cc == CB - 1),
                                skip_group_check=True)
                    if cc < CB - 1:
                        # keep PE busy (HAM warm) while the next chunk lands
                        for dd in range(2):
                            pd = ps_w.tile([128, 512], F32, tag="w",
                                           name=f"dumq{cc}_{dd}")
                            nc.tensor.matmul(pd[:], dummyS[:], dummyM[:],
                                             start=True, stop=True)
                nc.vector.tensor_copy(qT[0][:], s0[:])
                nc.vector.tensor_copy(kT[0][:], s1[:])

            def emit_v_group(ib, half):
                pv = ps_w.tile([128, 384], F32, tag="w", name=f"pv{ib}_{half}")
                for cc in range(CB):
                    nc.tensor.matmul(
                        pv[:],
                        xt[cc][:, ib * 128:(ib + 1) * 128],
                        wq[cc][:, 2 * C + half * 384: 2 * C + (half + 1) * 384],
                        start=(cc == 0), stop=(cc == CB - 1))
                nc.vector.tensor_copy(
                    vn[ib][:, half * 6 * VW:(half + 1) * 6 * VW]
                    .rearrange("p (h d) -> p h d", d=VW)[:, :, 0:HD],
                    pv[:].rearrange("p (h d) -> p h d", d=HD))
                if half == 1:
                    nc.vector.tensor_copy(
                        vn[ib][:].rearrange("p (h d) -> p h d", d=VW)[:, :, HD:VW],
                        ones12[:])

            ys = [None] * NB

            def emit_proj_half(nb, cp):
                py = ps_s.tile([128, 384], F32, tag="s", name=f"py{nb}_{cp}")
                for t2 in range(CB):
                    nc.tensor.matmul(
                        py[:], outT[t2][:, nb * 128:(nb + 1) * 128],
                        wp[t2][:, cp * 384:(cp + 1) * 384],
                        start=(t2 == 0), stop=(t2 == CB - 1))
                if ys[nb] is None:
                    ys[nb] = p_y.tile([128, C], F32, tag="ys", name=f"ys{nb}")
                nc.vector.tensor_add(
                    ys[nb][:, cp * 384:(cp + 1) * 384], py[:],
                    bias_bc[:, cp * 384:(cp + 1) * 384])

            def emit_y_dma(nb):
                nc.sync.dma_start(y.ap()[nb * 128:(nb + 1) * 128, :], ys[nb][:])

            # ---- prologue: qk for pairs 0,1 and v for blocks 0,1
            emit_qk0_ccprog()
            for f_idx in (0, 1):
                for nh in (0, 1):
                    emit_qk_quarter(1, f_idx, nh)
            for ib in (0, 1):
                for half in (0, 1):
                    emit_v_group(ib, half)

            # ---- weave plan
            weave = {}

            def add_weave(t, ih, jp, thunk):
                weave.setdefault((t, ih, jp), []).append(thunk)

            def pop_weave(t, ih, jp, n):
                lst = weave.get((t, ih, jp), [])
                for _ in range(n):
                    if not lst:
                        return
                    lst.pop(0)()

            # v blocks 2..7 early in (t=0, ih=0)
            for k in range(6):
                ib = k + 2
                for half in (0, 1):
                    add_weave(0, 0, min(3, k // 2), (lambda ib=ib, half=half:
                                                     emit_v_group(ib, half)))
            # qk pairs 2..5, four quarter-chunks each
            qslots = {
                2: [(0, 0, 2), (0, 0, 3), (0, 1, 2), (0, 1, 3)],
                3: [(1, 0, 2), (1, 0, 3), (1, 1, 2), (1, 1, 3)],
                4: [(2, 0, 2), (2, 0, 3), (2, 1, 2), (2, 1, 3)],
                5: [(3, 0, 2), (3, 0, 3), (3, 1, 2), (3, 1, 3)],
            }
            for t, slots in qslots.items():
                qi = 0
                for f_idx in (0, 1):
                    for nh in (0, 1):
                        ts_, ihs, jps = slots[qi]
                        add_weave(ts_, ihs, jps,
                                  (lambda t=t, f=f_idx, n=nh:
                                   emit_qk_quarter(t, f, n)))
                        qi += 1
            # proj for the ih=0 query half woven into (t=5, ih=1)
            for nb in range(4):
                for cp in (0, 1):
                    add_weave(5, 1, nb, (lambda nb=nb, cp=cp:
                                         emit_proj_half(nb, cp)))
                add_weave(5, 1, nb, (lambda nb=nb: emit_y_dma(nb)))

            # ---- attention: per head pair, PV lagged one jp
            for t in range(HP):
                hA, hB = 2 * t, 2 * t + 1
                for ih in (0, 1):
                    ppA = ps_pv.tile([VW, 512], F32, tag="pv", name=f"ppA{t}{ih}")
                    ppB = ps_pv.tile([VW, 512], F32, tag="pv", name=f"ppB{t}{ih}")

                    def emit_pv(jp, es):
                        # es[jbi] = [E_A(jb) | E_B(jb)] for jb = 2jp+jbi
                        for jbi, jb in enumerate((2 * jp, 2 * jp + 1)):
                            eU = es[jbi]
                            st = jp == 0 and jbi == 0
                            sp = jp == 3 and jbi == 1
                            nc.tensor.matmul(
                                ppA[:], vn[jb][:, hA * VW:(hA + 1) * VW],
                                eU[:, 0:512], start=st, stop=sp)
                            nc.tensor.matmul(
                                ppB[:], vn[jb][:, hB * VW:(hB + 1) * VW],
                                eU[:, 512:1024], start=st, stop=sp)

                    prev = None
                    for jp in range(4):
                        jbs = (2 * jp, 2 * jp + 1)
                        es = []
                        for jbi, jb in enumerate(jbs):
                            # pair writes the two banks of ONE tile
                            sU = ps_s.tile([128, 1024], F32, tag="s",
                                           name=f"sU{t}{ih}{jp}{jb}")
                            nc.tensor.matmul(
                                sU[:, 0:512],
                                kT[t][0:64, jb * 128:(jb + 1) * 128],
                                qT[t][0:64, ih * 512:(ih + 1) * 512],
                                start=True, stop=True, tile_position=(0, 0))
                            nc.tensor.matmul(
                                sU[:, 512:1024],
                                kT[t][64:128, jb * 128:(jb + 1) * 128],
                                qT[t][64:128, ih * 512:(ih + 1) * 512],
                                start=True, stop=True, tile_position=(64, 0))
                            eU = p_E.tile([128, 1024], BF16, tag="e",
                                          name=f"e{t}{ih}{jp}{jb}")
                            nc.scalar.activation(eU[:], sU[:], EXP, scale=SCALE)
                            es.append(eU)
                            if jbi == 0:
                                pop_weave(t, ih, jp, 1)
                        if prev is not None:
                            emit_pv(prev[0], prev[1])
                        prev = (jp, es)
                        pop_weave(t, ih, jp, 99)
                    emit_pv(prev[0], prev[1])

                    # ---- normalize: out^T = PV[0:64] / rowsum (row 64).
                    # Evacuate psum to SBUF first so the pv ring frees fast;
                    # the recip/mul chain then runs off the critical path.
                    for pp, po in ((ppA, 0), (ppB, 64)):
                        cp_ = p_nrm.tile([VW, 512], F32, tag=f"cp{po}",
                                         name=f"cp{t}{ih}{po}")
                        nc.vector.tensor_copy(cp_[:], pp[:])
                        rs = p_nrm.tile([1, 512], F32, tag=f"rs{po}",
                                        name=f"rs{t}{ih}{po}")
                        nc.vector.tensor_copy(rs[:], cp_[HD:VW, :])
                        bc = p_nrm.tile([64, 512], F32, tag=f"bc{po}",
                                        name=f"bc{t}{ih}{po}")
                        nc.gpsimd.partition_broadcast(bc[:], rs[:])
                        rc = p_nrm.tile([64, 512], F32, tag=f"rc{po}",
                                        name=f"rc{t}{ih}{po}")
                        nc.vector.reciprocal_approx_fast(rc[:], bc[:])
                        if po == 0:
                            nc.vector.tensor_mul(
                                outT[t][0:64, ih * 512:(ih + 1) * 512],
                                cp_[0:HD, :], rc[:])
                        else:
                            ob = p_nrm.tile([64, 512], BF16, tag="ob",
                                            name=f"ob{t}{ih}")
                            nc.vector.tensor_mul(ob[:], cp_[0:HD, :], rc[:])
                            nc.sync.dma_start(
                                outT[t][64:128, ih * 512:(ih + 1) * 512], ob[:])

            # ---- proj tail: ih=1 query half + remaining stores
            for nb in range(4, NB):
                for cp in (0, 1):
                    emit_proj_half(nb, cp)
                emit_y_dma(nb)

    nc.compile()
    return nc


def _get_nc():
    global _NC
    if _NC is None:
        _NC = _build()
    return _NC


def _run(in_maps, trace=False, tmpdir=None):
    return bass_utils.run_bass_kernel_spmd(
        _get_nc(), in_maps, core_ids=list(range(B)), trace=trace, tmpdir=tmpdir)


def _in_maps(x, w_qkv, w_proj, b_proj):
    import ml_dtypes
    bf = ml_dtypes.bfloat16
    x = np.asarray(x, dtype=np.float32).astype(bf)
    w_qkv = np.ascontiguousarray(np.asarray(w_qkv, dtype=np.float32).astype(bf))
    w_proj = np.ascontiguousarray(np.asarray(w_proj, dtype=np.float32).astype(bf))
    b_proj = np.ascontiguousarray(np.asarray(b_proj, dtype=np.float32)).reshape(1, C)
    return [
        {"x": np.ascontiguousarray(x[b].T), "w_qkv": w_qkv,
         "w_proj": w_proj, "b_proj": b_proj}
        for b in range(B)
    ]


def kernel(x, w_qkv, w_proj, b_proj):
    res = _run(_in_maps(x, w_qkv, w_proj, b_proj))
    return np.stack([res.results[b]["y"] for b in range(B)], axis=0)
